# revision 1
# baseline (speedup 1.0000x reference)
# MixGAT layer (GATConv + beta-mix swish) on 8 Trainium2 NeuronCores.
#
# Strategy (dst-node sharding, per spec sharding_hint):
#  - Nodes partitioned across 8 cores by dst id; each core owns N/8 dst rows.
#  - Params (W, att, bias) replicated; each core computes the full projected
#    table xp = x @ W on-device (replicated compute beats collectives here).
#  - Phase A (device): xp (fp16), a_src/a_dst (f32) written to a 512B/row HBM
#    table:  [128 x fp16 xp | 1.0 fp16 | 4 x f32 a_src | 4 x f32 a_dst | pad].
#  - Phase B (device): edges sorted by dst, packed into 32-node groups of
#    9 static 128-edge blocks (6 "lo" + 3 "hi": dma_gather indices are int16,
#    so the table is addressed as two halves).  dma_gather pulls each edge's
#    512B row (edge -> partition).  alpha = lrelu(a_src+a_dst) -> exp on ACT.
#    A sparse per-block weight matrix Mw[e, h*32+c] = exp(alpha[e,h]) (c = dst
#    slot) is built with is_equal/mult on DVE; ONE matmul per block computes
#    both the weighted feature aggregation and the softmax denominators (the
#    baked-in 1.0 column), accumulating over the group's blocks in PSUM.
#    Postproc divides by the denominator and applies the beta-mix swish.
#  - Phase C (device): small gather permutes group-slot rows to node order.
#
# kernel(**inputs) is self-contained: preprocessing is pure numpy, the device
# kernel is built with bass/Tile and run via run_bass_kernel_spmd on cores 0-7.

import math

import numpy as np

import concourse.bass as bass
import concourse.mybir as mybir
import concourse.tile as tile
from concourse import bacc
from concourse.bass_utils import run_bass_kernel_spmd

F32 = mybir.dt.float32
F16 = mybir.dt.float16
I16 = mybir.dt.int16

# problem constants
N_NODES = 50000
IN_DIM = 128
HEADS = 4
OUT_DIM = 32
LEAKY_SLOPE = 0.2
BETA = 0.5
CMIX = 1.2
N_CORES = 8

# static schedule constants
WIN = 32          # dst nodes per group (PSUM slots = HEADS*WIN = 128)
BLK = 128         # edges per block (gather slots -> partitions)
LOB = 5           # lo blocks per group
HIB = 3           # hi blocks per group
GPB = 4           # groups per superblock
SPLIT = 32768     # int16-addressable table split
LO_CAP = LOB * BLK
HI_CAP = HIB * BLK
DEAD = 100.0      # colidx value for dead slots (never equals iota 0..31)
ROW_F16 = 256     # fp16 elements per 512B table row
PC_CHUNK = 2048   # phase C gather chunk (nodes)
GNJ = 512        # max rows per dma_gather call
                  # for large gathers (SWDGE ring wrap), so split big gathers


def _gather_chunks(total):
    """Split `total` rows into even dma_gather chunks (multiples of 128)."""
    n = -(-total // GNJ)
    base = -(-(total // 128) // n) * 128
    out = []
    o = 0
    while o < total:
        c = min(base, total - o)
        out.append((o, c))
        o += c
    return out


class Cfg:
    def __init__(self, n_all, npc, split, nsb, ncc, pd=F16, bias_nonzero=False,
                 n_cores=N_CORES, phases="ABC", blevel=4, repeat=1):
        self.phases = phases
        self.blevel = blevel
        self.repeat = repeat     # bench-only: replicate the phase body
        self.n_all = n_all          # total nodes (table rows)
        self.npc = npc              # nodes per core
        self.split = split
        self.nsb = nsb              # superblocks per core
        self.ncc = ncc              # phase C chunks
        self.pd = pd
        self.bias_nonzero = bias_nonzero
        self.n_cores = n_cores


# ---------------------------------------------------------------- host side

def build_nc_adst(n_rows, pd, n_cores, repeat=1):
    """Launch-1 mini kernel: adstv[4, n_rows] = (W @ AD4).T @ xT_slab per core.
    x arrives pre-transposed ([feat, node]) so no on-device transpose."""
    nc = bacc.Bacc("TRN2", target_bir_lowering=False, debug=False,
                   num_devices=n_cores)
    TW = 512          # av_ps [4, TW] f32 must fit a 2KB PSUM bank
    xs_t = nc.dram_tensor("xT_slab", [IN_DIM, n_rows], pd, kind="ExternalInput")
    wad_t = nc.dram_tensor("wad_pd", [IN_DIM, HEADS], pd, kind="ExternalInput")
    out_t = nc.dram_tensor("adstv", [HEADS, n_rows], F32, kind="ExternalOutput")
    with tile.TileContext(nc) as tc:
        with (tc.tile_pool(name="c", bufs=1) as cp,
              tc.tile_pool(name="s", bufs=3) as sp,
              tc.tile_pool(name="p2", bufs=3, space="PSUM") as pp2):
            wad_c = cp.tile([IN_DIM, HEADS], pd)
            nc.sync.dma_start(wad_c[:], wad_t.ap())
            for _rep in range(repeat):
                for n0 in range(0, n_rows, TW):
                    p = min(TW, n_rows - n0)
                    xt8 = sp.tile([128, TW], pd, tag="xt")
                    nc.sync.dma_start(xt8[:, :p], xs_t.ap()[:, n0:n0 + p])
                    av_ps = pp2.tile([HEADS, TW], F32, tag="av")
                    nc.tensor.matmul(av_ps[:, :p], lhsT=wad_c[:], rhs=xt8[:, :p],
                                     start=True, stop=True)
                    av8 = sp.tile([HEADS, TW], F32, tag="av8")
                    nc.vector.tensor_copy(av8[:, :p], av_ps[:, :p])
                    nc.sync.dma_start(out_t.ap()[:, n0:n0 + p], av8[:, :p])
    nc.compile()
    return nc


def _wrap16(v):
    """idx vector [S*16] -> dma_gather idx layout [128, S]."""
    s = v.reshape(-1, 16).T                      # [16, S]
    return np.tile(s, (8, 1)).astype(np.int16)   # [128, S]


def preprocess(edge_index, n_all, npc, split, n_cores):
    """Build per-core static schedules. Returns (cfg-ish dict, per-core arrays)."""
    src = np.asarray(edge_index[0], dtype=np.int64)
    dst = np.asarray(edge_index[1], dtype=np.int64)
    loop = np.arange(n_all, dtype=np.int64)
    src = np.concatenate([src, loop])
    dst = np.concatenate([dst, loop])
    order = np.argsort(dst, kind="stable")
    src = src[order]
    dst = dst[order]

    core_bounds = np.searchsorted(dst, np.arange(n_cores + 1) * npc)
    cores = []
    for c in range(n_cores):
        b0, b1 = core_bounds[c], core_bounds[c + 1]
        s = src[b0:b1]
        d = (dst[b0:b1] - c * npc).astype(np.int64)
        lo_mask = s < split
        deg_lo = np.bincount(d[lo_mask], minlength=npc)
        deg_hi = np.bincount(d[~lo_mask], minlength=npc)
        # node -> edge range (d is sorted)
        seg_end = np.cumsum(np.bincount(d, minlength=npc))
        seg_start = seg_end - (deg_lo + deg_hi)

        # greedy 32-node groups under the static caps
        groups = []  # (n0, cnt)
        n = 0
        while n < npc:
            cnt, lo, hi = 0, 0, 0
            while (n + cnt < npc and cnt < WIN
                   and lo + deg_lo[n + cnt] <= LO_CAP
                   and hi + deg_hi[n + cnt] <= HI_CAP):
                lo += deg_lo[n + cnt]
                hi += deg_hi[n + cnt]
                cnt += 1
            assert cnt > 0, "single node exceeds block caps"
            groups.append((n, cnt))
            n += cnt
        g_cnt = len(groups)

        # per-group static block data
        idx_lo = np.zeros((g_cnt, LO_CAP), dtype=np.int64)
        idx_hi = np.zeros((g_cnt, HI_CAP), dtype=np.int64)
        col_lo = np.full((g_cnt, LO_CAP), DEAD, dtype=np.float16)
        col_hi = np.full((g_cnt, HI_CAP), DEAD, dtype=np.float16)
        dn_lo = np.zeros((g_cnt, LO_CAP), dtype=np.int64)
        dn_hi = np.zeros((g_cnt, HI_CAP), dtype=np.int64)
        perm = np.empty(npc, dtype=np.int64)
        for g, (n0, cnt) in enumerate(groups):
            e0, e1 = seg_start[n0], seg_end[n0 + cnt - 1]
            gs = s[e0:e1]
            gd = d[e0:e1] - n0
            m = gs < split
            nl = int(m.sum())
            nh = int((~m).sum())
            idx_lo[g, :nl] = gs[m]
            col_lo[g, :nl] = gd[m].astype(np.float16)
            dn_lo[g, :nl] = c * npc + n0 + gd[m]
            idx_hi[g, :nh] = gs[~m] - split
            col_hi[g, :nh] = gd[~m].astype(np.float16)
            dn_hi[g, :nh] = c * npc + n0 + gd[~m]
            perm[n0:n0 + cnt] = g * WIN + np.arange(cnt)
        cores.append(dict(g_cnt=g_cnt, idx_lo=idx_lo, idx_hi=idx_hi,
                          col_lo=col_lo, col_hi=col_hi, dn_lo=dn_lo,
                          dn_hi=dn_hi, perm=perm))

    g_max = max(c["g_cnt"] for c in cores)
    g_pad = ((g_max + GPB - 1) // GPB) * GPB
    nsb = g_pad // GPB
    ncc = (npc + PC_CHUNK - 1) // PC_CHUNK

    per_core = []
    for c in cores:
        g_cnt = c["g_cnt"]
        il = np.zeros((g_pad, LO_CAP), dtype=np.int64)
        ih = np.zeros((g_pad, HI_CAP), dtype=np.int64)
        cl = np.full((g_pad, LO_CAP), DEAD, dtype=np.float16)
        ch = np.full((g_pad, HI_CAP), DEAD, dtype=np.float16)
        dl = np.zeros((g_pad, LO_CAP), dtype=np.int64)
        dh = np.zeros((g_pad, HI_CAP), dtype=np.int64)
        il[:g_cnt] = c["idx_lo"]
        ih[:g_cnt] = c["idx_hi"]
        cl[:g_cnt] = c["col_lo"]
        ch[:g_cnt] = c["col_hi"]
        dl[:g_cnt] = c["dn_lo"]
        dh[:g_cnt] = c["dn_hi"]

        # gather idx tensors: [NSB, 128, S]
        idx_lo_t = np.stack([_wrap16(il[sb * GPB:(sb + 1) * GPB].reshape(-1))
                             for sb in range(nsb)])
        idx_hi_t = np.stack([_wrap16(ih[sb * GPB:(sb + 1) * GPB].reshape(-1))
                             for sb in range(nsb)])

        # colidx tensor: [NSB, 128, GPB*(LOB+HIB)]  (block cols: 24 lo | 12 hi)
        cl_b = cl.reshape(g_pad, LOB, BLK)
        ch_b = ch.reshape(g_pad, HIB, BLK)
        dl_b = dl.reshape(g_pad, LOB, BLK)
        dh_b = dh.reshape(g_pad, HIB, BLK)
        colidx = np.empty((nsb, 128, GPB * (LOB + HIB)), dtype=np.float16)
        dstn = np.empty((nsb, 128, GPB * (LOB + HIB)), dtype=np.int64)
        for sb in range(nsb):
            sl = slice(sb * GPB, (sb + 1) * GPB)
            colidx[sb, :, :GPB * LOB] = cl_b[sl].reshape(-1, BLK).T
            colidx[sb, :, GPB * LOB:] = ch_b[sl].reshape(-1, BLK).T
            dstn[sb, :, :GPB * LOB] = dl_b[sl].reshape(-1, BLK).T
            dstn[sb, :, GPB * LOB:] = dh_b[sl].reshape(-1, BLK).T

        # phase C perm idx: [NCC, 128, PC_CHUNK//16]
        permidx = np.empty((ncc, 128, PC_CHUNK // 16), dtype=np.int16)
        for k in range(ncc):
            v = np.full(PC_CHUNK, -1, dtype=np.int64)
            n0 = k * PC_CHUNK
            n1 = min(npc, n0 + PC_CHUNK)
            v[:n1 - n0] = c["perm"][n0:n1]
            permidx[k] = _wrap16(v)

        per_core.append(dict(idx_lo=idx_lo_t, idx_hi=idx_hi_t, colidx=colidx,
                             dstn=dstn, permidx=permidx))
    return nsb, ncc, g_pad, per_core


# -------------------------------------------------------------- device side

def build_nc(cfg: Cfg):
    nc = bacc.Bacc("TRN2", target_bir_lowering=False, debug=False,
                   num_devices=cfg.n_cores)
    PD = cfg.pd
    n_all, npc, split, nsb, ncc = cfg.n_all, cfg.npc, cfg.split, cfg.nsb, cfg.ncc
    NBLK = GPB * (LOB + HIB)          # blocks per superblock (36)
    NLO = GPB * LO_CAP                # lo idxs per superblock (3072)
    NHI = GPB * HI_CAP                # hi idxs per superblock
    g_pad = nsb * GPB

    WF = IN_DIM + 1 + 2 * HEADS      # [W | zero col | W@att_src | W@att_dst]
    x_t = nc.dram_tensor("xT_pd", [IN_DIM, n_all], PD, kind="ExternalInput")
    wf_t = nc.dram_tensor("wfull_pd", [IN_DIM, WF], PD, kind="ExternalInput")
    iota_t = nc.dram_tensor("iota_pd", [128, WIN], PD, kind="ExternalInput")
    biasb_t = nc.dram_tensor("biasb", [128, HEADS * OUT_DIM], F32, kind="ExternalInput")
    SWT = NLO // 16 + NHI // 16 + NBLK + NBLK * HEADS
    st_t = nc.dram_tensor("streams", [nsb, 128, SWT], I16, kind="ExternalInput")
    pi_t = nc.dram_tensor("permidx", [ncc, 128, PC_CHUNK // 16], I16, kind="ExternalInput")
    out_t = nc.dram_tensor("out", [npc, HEADS * OUT_DIM], F32, kind="ExternalOutput")

    table = nc.dram_tensor("table", [n_all, ROW_F16], PD, kind="Internal")
    scratch = nc.dram_tensor("scratch", [g_pad * WIN, 128], PD, kind="Internal")

    with tile.TileContext(nc) as tc:
        with tc.tile_pool(name="consts", bufs=1) as cpool:
            wf_c = cpool.tile([IN_DIM, WF], PD)
            nc.sync.dma_start(wf_c[:], wf_t.ap())
            iota_c = cpool.tile([128, WIN], PD)
            nc.sync.dma_start(iota_c[:], iota_t.ap())
            biasb_c = cpool.tile([128, HEADS * OUT_DIM], F32)
            nc.sync.dma_start(biasb_c[:], biasb_t.ap())

            for _rep in range(cfg.repeat):
                # ---------------- phase A: projection table ----------------
                # x arrives pre-transposed ([feat, node]) so each 128-node tile is
                # ONE matmul (lhsT = xT slice): out[n, :] = [xp | 0 | a_src a_dst]
                # (a_src/a_dst as f16 at cols 129:137).  8 tiles are batched per
                # x-load / table-write DMA: each dma_start holds the shared HWDGE
                # descriptor unit ~625ns, so DMA instruction count rules.
                n_tiles = (n_all + 127) // 128 if "A" in cfg.phases else 0
                T8 = 8
                n_big = n_all // (128 * T8) if "A" in cfg.phases else 0
                with (tc.tile_pool(name="pa_sb", bufs=3) as pa,
                      tc.tile_pool(name="pa_ps2", bufs=4, space="PSUM") as pps2):
                    for it in range(n_big):
                        n0 = it * 128 * T8
                        xt8 = pa.tile([128, T8 * 128], PD, tag="xt")
                        nc.sync.dma_start(xt8[:], x_t.ap()[:, n0:n0 + 128 * T8])
                        rows8 = pa.tile([128, T8, ROW_F16], PD, tag="rowsb")
                        for j in range(T8):
                            ps = pps2.tile([128, WF], F32, tag="ps")
                            nc.tensor.matmul(ps[:],
                                             lhsT=xt8[:, j * 128:(j + 1) * 128],
                                             rhs=wf_c[:], start=True, stop=True)
                            nc.vector.tensor_copy(rows8[:, j, 0:WF], ps[:])
                        nc.vector.memset(rows8[:, :, 128:129], 1.0)
                        nc.sync.dma_start(
                            table.ap()[n0:n0 + 128 * T8, :]
                                 .rearrange("(j p) f -> p j f", p=128),
                            rows8[:])
                    for t in range(n_big * T8, n_tiles):
                        n0 = t * 128
                        p = min(128, n_all - n0)
                        xt = pa.tile([128, 128], PD, tag="xtt")
                        nc.sync.dma_start(xt[:, :p], x_t.ap()[:, n0:n0 + p])
                        ps = pps2.tile([128, WF], F32, tag="ps")
                        nc.tensor.matmul(ps[:p, :], lhsT=xt[:, :p], rhs=wf_c[:],
                                         start=True, stop=True)
                        rows = pa.tile([128, ROW_F16], PD, tag="rowst")
                        nc.vector.tensor_copy(rows[:p, 0:WF], ps[:p, :])
                        nc.vector.memset(rows[:p, 128:129], 1.0)
                        nc.sync.dma_start(table.ap()[n0:n0 + p, :], rows[:p, :])

                # ---------------- phase B: gather + aggregate ----------------
                with (tc.tile_pool(name="pb_g", bufs=3) as gp,
                      tc.tile_pool(name="pb_m", bufs=3) as mp,
                      tc.tile_pool(name="pb_s", bufs=4) as sp,
                      tc.tile_pool(name="pb_z", bufs=3) as zp,
                      tc.tile_pool(name="pb_ps", bufs=8, space="PSUM") as up):
                    BL = cfg.blevel
                    SW0 = NLO // 16
                    SW1 = SW0 + NHI // 16
                    SW2 = SW1 + NBLK
                    SW = SW2 + NBLK * HEADS
                    for sb in range(nsb if "B" in cfg.phases else 0):
                        strm = sp.tile([128, SW], I16, tag="strm")
                        nc.sync.dma_start(strm[:], st_t.ap()[sb])
                        il = strm[:, 0:SW0]
                        ih = strm[:, SW0:SW1]
                        cx = strm[:, SW1:SW2].bitcast(F16)
                        adp = (strm[:, SW2:SW].bitcast(F16)
                               .rearrange("p (b h) -> p b h", h=HEADS))

                        glo = gp.tile([128, GPB * LOB, ROW_F16], PD, tag="glo")
                        for j0, nj in _gather_chunks(NLO):
                            nc.gpsimd.dma_gather(
                                glo[:, j0 // 128:(j0 + nj) // 128, :],
                                table.ap()[0:split, :],
                                il[:, j0 // 16:(j0 + nj) // 16],
                                nj, nj, ROW_F16, single_packet=False)
                        ghi = gp.tile([128, GPB * HIB, ROW_F16], PD, tag="ghi")
                        for j0, nj in _gather_chunks(NHI):
                            nc.gpsimd.dma_gather(
                                ghi[:, j0 // 128:(j0 + nj) // 128, :],
                                table.ap()[split:n_all, :],
                                ih[:, j0 // 16:(j0 + nj) // 16],
                                nj, nj, ROW_F16, single_packet=False)

                        if BL < 2:
                            continue
                        # onehot[e, b, c] = (iota[c] == colidx[e, b])
                        oneh = mp.tile([128, NBLK, WIN], PD, tag="oneh")
                        nc.vector.tensor_tensor(
                            out=oneh[:],
                            in0=iota_c[:].unsqueeze(1).to_broadcast([128, NBLK, WIN]),
                            in1=cx.unsqueeze(2).to_broadcast([128, NBLK, WIN]),
                            op=mybir.AluOpType.is_equal)

                        # alpha = a_src[src] + a_dst[dst] ; lrelu ; exp
                        asum = sp.tile([128, NBLK, HEADS], F32, tag="asum")
                        nc.vector.tensor_tensor(out=asum[:, :GPB * LOB, :],
                                                in0=glo[:, :, 129:129 + HEADS],
                                                in1=adp[:, :GPB * LOB, :],
                                                op=mybir.AluOpType.add)
                        nc.vector.tensor_tensor(out=asum[:, GPB * LOB:, :],
                                                in0=ghi[:, :, 129:129 + HEADS],
                                                in1=adp[:, GPB * LOB:, :],
                                                op=mybir.AluOpType.add)
                        asc = sp.tile([128, NBLK, HEADS], F32, tag="asc")
                        nc.vector.tensor_scalar(asc[:], asum[:], LEAKY_SLOPE, None,
                                                mybir.AluOpType.mult)
                        alr = sp.tile([128, NBLK, HEADS], F32, tag="alr")
                        nc.vector.tensor_tensor(out=alr[:], in0=asum[:], in1=asc[:],
                                                op=mybir.AluOpType.max)
                        expa = sp.tile([128, NBLK, HEADS], PD, tag="expa")
                        nc.scalar.activation(expa[:], alr[:],
                                             mybir.ActivationFunctionType.Exp)

                        # Mw[e, b, h*32+c] = oneh[e, b, c] * expa[e, b, h]
                        mw = mp.tile([128, NBLK, HEADS, WIN], PD, tag="mw")
                        nc.vector.tensor_tensor(
                            out=mw[:],
                            in0=oneh[:].unsqueeze(2).to_broadcast([128, NBLK, HEADS, WIN]),
                            in1=expa[:].unsqueeze(3).to_broadcast([128, NBLK, HEADS, WIN]),
                            op=mybir.AluOpType.mult)

                        if BL < 3:
                            continue
                        zn_all = zp.tile([128, GPB, 128], PD, tag="znall")
                        for g in range(GPB):
                            U = up.tile([128, 130], F32, tag="U")
                            for j in range(LOB + HIB):
                                if j < LOB:
                                    col = g * LOB + j
                                    rhs = glo[:, col, 0:129]
                                else:
                                    col = GPB * LOB + g * HIB + (j - LOB)
                                    rhs = ghi[:, col - GPB * LOB, 0:129]
                                nc.tensor.matmul(U[:, 0:129], lhsT=mw[:, col, :, :],
                                                 rhs=rhs, start=(j == 0),
                                                 stop=(j == LOB + HIB - 1))
                            if BL < 4:
                                continue
                            # softmax-normalize only; swish moves to phase C
                            # (node space = 4x fewer elements, idle engines there)
                            rec = zp.tile([128, 1], F32, tag="rec")
                            nc.vector.reciprocal(rec[:], U[:, 128:129])
                            nc.scalar.activation(zn_all[:, g, :], U[:, 0:128],
                                                 mybir.ActivationFunctionType.Copy,
                                                 scale=rec[:, 0:1])
                        if BL < 4:
                            continue
                        # Per-head diagonal extraction on DVE, then ONE batched
                        # scratch write for the whole superblock (128 node rows).
                        nz2 = zp.tile([WIN, GPB, HEADS, WIN], PD, tag="nz2")
                        for h in range(HEADS):
                            nc.vector.tensor_copy(
                                nz2[:, :, h, :],
                                zn_all[h * WIN:(h + 1) * WIN, :,
                                       h * WIN:(h + 1) * WIN])
                        r0 = sb * GPB * WIN
                        nc.sync.dma_start(
                            scratch.ap()[r0:r0 + GPB * WIN, :]
                                   .rearrange("(g c) (h k) -> c g h k",
                                              c=WIN, k=WIN),
                            nz2[:])

                # ------- phase C: permute to node order + beta-mix swish -------
                with (tc.tile_pool(name="pc_s", bufs=2) as pcs,
                      tc.tile_pool(name="pc_g", bufs=2) as pcg):
                    for k in range(ncc if "C" in cfg.phases else 0):
                        n0 = k * PC_CHUNK
                        valid = min(PC_CHUNK, npc - n0)
                        pidx = pcs.tile([128, PC_CHUNK // 16], I16, tag="pidx")
                        nc.sync.dma_start(pidx[:], pi_t.ap()[k])
                        gt = pcg.tile([128, PC_CHUNK // 128, 128], PD, tag="gt")
                        for j0, nj in _gather_chunks(PC_CHUNK):
                            nc.gpsimd.dma_gather(
                                gt[:, j0 // 128:(j0 + nj) // 128, :],
                                scratch.ap()[:, :],
                                pidx[:, j0 // 16:(j0 + nj) // 16],
                                nj, min(nj, max(0, valid - j0)), 128,
                                single_packet=False)
                        if cfg.bias_nonzero:
                            nc.vector.tensor_tensor(
                                out=gt[:], in0=gt[:],
                                in1=biasb_c[:].unsqueeze(1)
                                    .to_broadcast([128, PC_CHUNK // 128, 128]),
                                op=mybir.AluOpType.add)
                        sg = pcg.tile([128, PC_CHUNK // 128, 128], PD, tag="sg")
                        nc.scalar.activation(sg[:], gt[:],
                                             mybir.ActivationFunctionType.Sigmoid)
                        mix = pcg.tile([128, PC_CHUNK // 128, 128], PD, tag="mix")
                        nc.vector.tensor_scalar(mix[:], sg[:], CMIX - BETA, BETA,
                                                mybir.AluOpType.mult,
                                                mybir.AluOpType.add)
                        orow = pcg.tile([128, PC_CHUNK // 128, 128], F32, tag="or")
                        nc.vector.tensor_tensor(out=orow[:], in0=gt[:], in1=mix[:],
                                                op=mybir.AluOpType.mult)
                        nb = valid // 128
                        if nb:
                            nc.sync.dma_start(
                                out_t.ap()[n0:n0 + nb * 128, :]
                                     .rearrange("(b p) f -> p b f", p=128),
                                orow[:, 0:nb, :])
                        rem = valid - nb * 128
                        if rem:
                            nc.sync.dma_start(
                                out_t.ap()[n0 + nb * 128:n0 + valid, :],
                                orow[0:rem, nb, :])

    nc.compile()
    return nc


# ---------------------------------------------------------------- the API

def _make_const_inputs(W, att_src, att_dst, bias, pd_np):
    W = np.asarray(W, dtype=np.float32)
    att_src = np.asarray(att_src, dtype=np.float32)
    att_dst = np.asarray(att_dst, dtype=np.float32)
    bias = np.asarray(bias, dtype=np.float32)
    H, D = att_src.shape
    as4 = np.zeros((H * D, 2 * H), dtype=np.float32)
    for h in range(H):
        as4[h * D:(h + 1) * D, h] = att_src[h]
        as4[h * D:(h + 1) * D, H + h] = att_dst[h]
    wfull = np.zeros((H * D, H * D + 1 + 2 * H), dtype=np.float32)
    wfull[:, 0:H * D] = W
    wfull[:, H * D + 1:] = W @ as4        # param-only host matmul
    iota = np.tile(np.arange(WIN, dtype=np.float32), (128, 1))
    biasb = np.tile(bias, (128, 1)).astype(np.float32)
    return dict(wfull_pd=wfull.astype(pd_np),
                iota_pd=iota.astype(pd_np), biasb=biasb)


def expand_adst(adstv, per_core, pd_np):
    """adstv [H, N] (device-computed) -> per-core adst_pe streams (host
    indexing only, no arithmetic)."""
    outs = []
    for c in per_core:
        dn = c["dstn"]                                  # [nsb, 128, NBLK]
        a = adstv[:, dn]                                # [H, nsb, 128, NBLK]
        a = np.moveaxis(a, 0, -1)                       # [nsb, 128, NBLK, H]
        nsb = a.shape[0]
        outs.append(np.ascontiguousarray(
            a.reshape(nsb, 128, -1).astype(pd_np)))
    return outs


def run(x, edge_index, W, att_src, att_dst, bias,
        n_all=N_NODES, n_cores=N_CORES, split=SPLIT, pd=F16, trace=False):
    npc = n_all // n_cores
    nsb, ncc, g_pad, per_core = preprocess(edge_index, n_all, npc, split, n_cores)
    bias_nonzero = bool(np.any(np.asarray(bias)))
    cfg = Cfg(n_all, npc, split, nsb, ncc, pd=pd, bias_nonzero=bias_nonzero,
              n_cores=n_cores)

    if pd == F16:
        pd_np = np.float16
    else:
        import ml_dtypes
        pd_np = ml_dtypes.bfloat16
    consts = _make_const_inputs(W, att_src, att_dst, bias, pd_np)
    xT16 = np.ascontiguousarray(
        np.asarray(x, dtype=np.float32).astype(pd_np).T)   # [feat, node]
    W32 = np.asarray(W, dtype=np.float32)
    att_dst32 = np.asarray(att_dst, dtype=np.float32)
    H, D = att_dst32.shape
    ad4 = np.zeros((H * D, H), dtype=np.float32)
    for h in range(H):
        ad4[h * D:(h + 1) * D, h] = att_dst32[h]
    wad = (W32 @ ad4).astype(pd_np)                     # param-only host matmul

    # launch 1: per-core a_dst slab
    nc1 = build_nc_adst(npc, pd, n_cores)
    in_maps1 = [dict(xT_slab=np.ascontiguousarray(xT16[:, c * npc:(c + 1) * npc]),
                     wad_pd=wad)
                for c in range(n_cores)]
    res1 = run_bass_kernel_spmd(nc1, in_maps1, core_ids=list(range(n_cores)),
                                trace=trace)
    adstv = np.concatenate([res1.results[c]["adstv"] for c in range(n_cores)],
                           axis=1)                      # [H, n_all]
    adst_pes = expand_adst(adstv, per_core, pd_np)

    # launch 2: the full layer
    nc = build_nc(cfg)
    in_maps = []
    for c in range(n_cores):
        m = dict(consts)
        m["xT_pd"] = xT16
        m["streams"] = np.ascontiguousarray(np.concatenate(
            [per_core[c]["idx_lo"], per_core[c]["idx_hi"],
             per_core[c]["colidx"].view(np.int16),
             adst_pes[c].view(np.int16)], axis=2))
        m["permidx"] = per_core[c]["permidx"]
        in_maps.append(m)
    res = run_bass_kernel_spmd(nc, in_maps, core_ids=list(range(n_cores)),
                               trace=trace)
    out = np.concatenate([res.results[c]["out"] for c in range(n_cores)], axis=0)
    parts = dict(nc1=nc1, in_maps1=in_maps1, nc2=nc, in_maps2=in_maps,
                 res1=res1, res2=res, n_cores=n_cores, cfg=cfg, npc=npc, pd=pd)
    return out, parts


def make_pjrt_fn(nc, in_maps, n_cores):
    """Build a jitted PJRT executor for a prebuilt Bass module (axon path).
    Returns (fn, args); inputs are pre-staged on device."""
    import jax
    from jax.sharding import Mesh, NamedSharding, PartitionSpec
    from jax.experimental.shard_map import shard_map

    import concourse.mybir as mybir_
    from concourse import bass2jax as b2j

    b2j.install_neuronx_cc_hook()
    partition_name = (nc.partition_id_tensor.name
                      if nc.partition_id_tensor else None)
    in_names, out_names, out_avals, zero_outs = [], [], [], []
    for alloc in nc.m.functions[0].allocations:
        if not isinstance(alloc, mybir_.MemoryLocationSet):
            continue
        name = alloc.memorylocations[0].name
        if alloc.kind == "ExternalInput":
            if name != partition_name:
                in_names.append(name)
        elif alloc.kind == "ExternalOutput":
            dt = mybir_.dt.np(alloc.dtype)
            out_avals.append(jax.core.ShapedArray(tuple(alloc.tensor_shape), dt))
            out_names.append(name)
            zero_outs.append(np.zeros(tuple(alloc.tensor_shape), dt))

    # the bind's in_names must cover ALL operands (inputs + zero-out bufs
    # + partition id) — neuronx_cc_hook asserts len(in_names) == n_operands.
    bind_names = list(in_names) + list(out_names)
    if partition_name is not None:
        bind_names.append(partition_name)

    def _body(*args):
        operands = list(args)
        if partition_name is not None:
            operands.append(b2j.partition_id_tensor())
        outs = b2j._bass_exec_p.bind(
            *operands, out_avals=tuple(out_avals), in_names=tuple(bind_names),
            out_names=tuple(out_names), lowering_input_output_aliases=(),
            sim_require_finite=True, sim_require_nnan=True, nc=nc)
        return tuple(outs)

    n_params = len(in_names)
    devices = jax.devices()[:n_cores]
    mesh = Mesh(np.asarray(devices), ("core",))
    spec = PartitionSpec("core")
    fn = jax.jit(shard_map(_body, mesh=mesh,
                           in_specs=(spec,) * (n_params + len(zero_outs)),
                           out_specs=(spec,) * len(out_names),
                           check_rep=False), keep_unused=True)
    sh = NamedSharding(mesh, spec)
    args = [jax.device_put(
                np.concatenate([in_maps[c][nm] for c in range(n_cores)], 0), sh)
            for nm in in_names]
    args += [jax.device_put(
                np.zeros((n_cores * z.shape[0], *z.shape[1:]), z.dtype), sh)
             for z in zero_outs]
    return fn, args


def bench_pair(fnA, argsA, fnB, argsB, iters=24):
    """Interleaved wall-clock of two executables with a 4-byte D2H fetch as
    the completion sync (block_until_ready alone is lost in ~40-90ms axon RPC
    noise; interleaving + cluster-min cancels the shared offset)."""
    import time as _time
    for fn, args in ((fnA, argsA), (fnB, argsB)):
        r = fn(*args)
        _ = np.asarray(r[0][0:1, 0:1])
    pa, pb = [], []
    for _ in range(iters):
        t0 = _time.perf_counter()
        r = fnA(*argsA)
        _ = np.asarray(r[0][0:1, 0:1])
        pa.append(_time.perf_counter() - t0)
        t0 = _time.perf_counter()
        r = fnB(*argsB)
        _ = np.asarray(r[0][0:1, 0:1])
        pb.append(_time.perf_counter() - t0)
    return np.array(pa), np.array(pb)


def bench_slope(ncA, ncB, in_maps, n_cores, reps, iters=24):
    """HW ns of one kernel body via the repeat-slope method: ncA has repeat=1,
    ncB has repeat=reps; returns (per-rep seconds, raw pair arrays)."""
    fnA, argsA = make_pjrt_fn(ncA, in_maps, n_cores)
    fnB, argsB = make_pjrt_fn(ncB, in_maps, n_cores)
    pa, pb = bench_pair(fnA, argsA, fnB, argsB, iters=iters)
    # medians: robust against the occasional anomalous fast RPC round-trip
    per = (np.median(pb) - np.median(pa)) / (reps - 1)
    return max(0.0, per), (pa, pb)


def kernel(**inputs) -> np.ndarray:
    out, _ = run(inputs["x"], inputs["edge_index"], inputs["W"],
                 inputs["att_src"], inputs["att_dst"], inputs["bias"])
    return out



# revision 2
# speedup vs baseline: 1.2353x; 1.2353x over previous
# MixGAT layer (GATConv + beta-mix swish) on 8 Trainium2 NeuronCores, v2.
#
# Strategy (dst-node sharding):
#  - Nodes partitioned across 8 cores by dst id; each core owns N/8 dst rows.
#  - KEY CHANGE vs v1: aggregation is linear in xp = x @ W, so we aggregate
#    RAW x features per dst and project ONCE per dst node afterwards:
#      out[d] = (softmax-weighted-sum_e x[src_e]) / denom @ W
#    This removes the on-device projection-table build (old phase A): the
#    gather table is just x cast to f16 on the host (node-major, 256B rows
#    instead of 512B).
#  - Launch 1 (tiny): per-node attention stats a_src/a_dst = x @ (W@att) on
#    device; host expands them into per-edge streams (indexing only).
#  - Launch 2, per superblock (128 dst nodes = 4 fixed 32-node groups):
#      dma_gather x rows per edge slot (lo/hi int16-index split, 256B rows,
#      single-packet descriptors spread over 4 SWDGE queues),
#      expa = exp(lrelu(a_src+a_dst)) from streams,
#      Mw[e, h*32+c] = expa[e, h] * onehot(dst slot c),
#      per 128-edge block:  U2[feat, slot] += glo_blk(lhsT) @ Mw(rhs)
#                           Dt[slot, 1]   += Mw(lhsT) @ ones(rhs)
#      per group: project  Z[c, h*32+o] = U2[:, h*32..](lhsT) @ W[:, h*32..]
#      per sb: denominators to [c, g, h] via 4 partition-shifted copies,
#      normalize, beta-mix swish, ONE node-ordered output DMA.
#    Fixed 32-node groups keep outputs contiguous: no scratch roundtrip and
#    no permutation pass. Per-group block counts are padded to the max over
#    cores so one SPMD module serves all 8 cores.
#
# kernel(**inputs) is self-contained: preprocessing is pure numpy (sorting /
# indexing / dtype casts only), device kernels built with bass/Tile, run via
# run_bass_kernel_spmd on cores 0-7.

import numpy as np

import concourse.bass as bass
import concourse.mybir as mybir
import concourse.tile as tile
from concourse import bacc
from concourse.bass_utils import run_bass_kernel_spmd

F32 = mybir.dt.float32
F16 = mybir.dt.float16
I16 = mybir.dt.int16

# problem constants
N_NODES = 50000
IN_DIM = 128
HEADS = 4
OUT_DIM = 32
LEAKY_SLOPE = 0.2
BETA = 0.5
CMIX = 1.2
N_CORES = 8

# static schedule constants
WIN = 32          # dst nodes per group (PSUM slots = HEADS*WIN = 128)
BLK = 128         # edges per block (gather slots -> partitions)
GPB = 4           # groups per superblock (4*32 = 128 dst nodes)
SPLIT = 32768     # int16-addressable table split
DEAD = 100.0      # colidx value for dead slots (never equals iota 0..31)
GNJ = 896         # rows per dma_gather call (SWDGE ring holds scratch/16
                  # descriptors; stay under 1024 at 1 desc/row)
NQ = 4            # SWDGE queues to spread gathers over
SCRATCH = 16384   # dynamic dma scratch (ring) bytes per partition
SINGLE_PACKET = True


def _wrap16(v):
    """idx vector [S*16] -> dma_gather idx layout [128, S]."""
    s = v.reshape(-1, 16).T                      # [16, S]
    return np.tile(s, (8, 1)).astype(np.int16)   # [128, S]


def _gather_chunks(total, gnj):
    out = []
    o = 0
    while o < total:
        c = min(gnj, total - o)
        out.append((o, c))
        o += c
    return out


class Cfg:
    def __init__(self, npc, n_cores=N_CORES, bias_nonzero=False, repeat=1,
                 blevel=4, gnj=GNJ, nq=NQ, scratch=SCRATCH, sp=SINGLE_PACKET):
        self.npc = npc
        self.n_cores = n_cores
        self.bias_nonzero = bias_nonzero
        self.repeat = repeat
        self.blevel = blevel   # 1 gather only; 2 +mw; 3 +matmul; 4 full
        self.gnj = gnj
        self.nq = nq
        self.scratch = scratch
        self.sp = sp


# ---------------------------------------------------------------- host side

def build_nc_stats(n_rows, n_cores, repeat=1):
    """Launch-1 mini kernel: statv[8, n_rows] = (W@[as|ad]).T @ xT_slab."""
    nc = bacc.Bacc("TRN2", target_bir_lowering=False, debug=False,
                   num_devices=n_cores)
    TW = 512
    H2 = 2 * HEADS
    xs_t = nc.dram_tensor("xT_slab", [IN_DIM, n_rows], F16, kind="ExternalInput")
    wad_t = nc.dram_tensor("wad_pd", [IN_DIM, H2], F16, kind="ExternalInput")
    out_t = nc.dram_tensor("statv", [H2, n_rows], F32, kind="ExternalOutput")
    with tile.TileContext(nc) as tc:
        with (tc.tile_pool(name="c", bufs=1) as cp,
              tc.tile_pool(name="s", bufs=3) as sp,
              tc.tile_pool(name="p2", bufs=3, space="PSUM") as pp2):
            wad_c = cp.tile([IN_DIM, H2], F16)
            nc.sync.dma_start(wad_c[:], wad_t.ap())
            for _rep in range(repeat):
                for n0 in range(0, n_rows, TW):
                    p = min(TW, n_rows - n0)
                    xt8 = sp.tile([128, TW], F16, tag="xt")
                    nc.sync.dma_start(xt8[:, :p], xs_t.ap()[:, n0:n0 + p])
                    av_ps = pp2.tile([H2, TW], F32, tag="av")
                    nc.tensor.matmul(av_ps[:, :p], lhsT=wad_c[:], rhs=xt8[:, :p],
                                     start=True, stop=True)
                    av8 = sp.tile([H2, TW], F32, tag="av8")
                    nc.vector.tensor_copy(av8[:, :p], av_ps[:, :p])
                    nc.sync.dma_start(out_t.ap()[:, n0:n0 + p], av8[:, :p])
    nc.compile()
    return nc


def preprocess(edge_index, n_all, npc, n_cores):
    """Static schedules: fixed 32-node groups, per-group block counts padded
    to the max over cores (one SPMD module). Pure numpy indexing."""
    src = np.asarray(edge_index[0], dtype=np.int64)
    dst = np.asarray(edge_index[1], dtype=np.int64)
    loop = np.arange(n_all, dtype=np.int64)
    src = np.concatenate([src, loop])
    dst = np.concatenate([dst, loop])
    order = np.argsort(dst, kind="stable")
    src = src[order]
    dst = dst[order]

    n_grp = (npc + WIN - 1) // WIN
    g_pad = ((n_grp + GPB - 1) // GPB) * GPB
    nsb = g_pad // GPB
    pad_n = g_pad * WIN - npc
    core_bounds = np.searchsorted(dst, np.arange(n_cores + 1) * npc)

    # stage A: per (core, group) lo/hi edge arrays
    per_cg = []
    for c in range(n_cores):
        b0, b1 = core_bounds[c], core_bounds[c + 1]
        s = src[b0:b1]
        d = (dst[b0:b1] - c * npc).astype(np.int64)
        if pad_n:  # virtual degree-1 edges for tail dead slots
            s = np.concatenate([s, np.zeros(pad_n, dtype=np.int64)])
            d = np.concatenate([d, np.arange(npc, npc + pad_n, dtype=np.int64)])
        gb = np.searchsorted(d // WIN, np.arange(g_pad + 1))
        rows = []
        for g in range(g_pad):
            e0, e1 = gb[g], gb[g + 1]
            gs = s[e0:e1]
            gc = (d[e0:e1] - g * WIN)
            gdst = np.minimum(c * npc + d[e0:e1], n_all - 1)
            m = gs < SPLIT
            rows.append(((gs[m], gc[m], gdst[m]),
                         (gs[~m] - SPLIT, gc[~m], gdst[~m])))
        per_cg.append(rows)

    # stage B: global per-group block counts (max over cores)
    nlo_g = [max((len(per_cg[c][g][0][0]) + BLK - 1) // BLK
                 for c in range(n_cores)) for g in range(g_pad)]
    nhi_g = [max((len(per_cg[c][g][1][0]) + BLK - 1) // BLK
                 for c in range(n_cores)) for g in range(g_pad)]
    sched = []
    for sb in range(nsb):
        gs = range(sb * GPB, (sb + 1) * GPB)
        sched.append(([nlo_g[g] for g in gs], [nhi_g[g] for g in gs]))

    # stage C: per-core padded stream arrays
    def pad_block(vals, nblk, fill, dtype):
        a = np.full(nblk * BLK, fill, dtype=dtype)
        a[:len(vals)] = vals
        return a

    cores = []
    for c in range(n_cores):
        sbs = []
        for sb in range(nsb):
            gl = range(sb * GPB, (sb + 1) * GPB)
            idx_parts, col_parts, src_parts, dst_parts = [], [], [], []
            for half in (0, 1):
                cnt_g = nlo_g if half == 0 else nhi_g
                for g in gl:
                    hs, hc, hd = per_cg[c][g][half]
                    nb = cnt_g[g]
                    if nb == 0:
                        continue
                    idx_parts.append((half, pad_block(hs, nb, 0, np.int64)))
                    col_parts.append(pad_block(hc.astype(np.float16), nb,
                                               DEAD, np.float16))
                    src_parts.append(pad_block(
                        hs + (0 if half == 0 else SPLIT), nb, 0, np.int64))
                    dst_parts.append(pad_block(hd, nb, 0, np.int64))
            lo_idx = np.concatenate([a for h, a in idx_parts if h == 0]) \
                if any(h == 0 for h, _ in idx_parts) else np.zeros(0, np.int64)
            hi_idx = np.concatenate([a for h, a in idx_parts if h == 1]) \
                if any(h == 1 for h, _ in idx_parts) else np.zeros(0, np.int64)
            colidx = np.concatenate(col_parts).reshape(-1, BLK)   # [nbk,128]
            srcid = np.concatenate(src_parts).reshape(-1, BLK)
            dstid = np.concatenate(dst_parts).reshape(-1, BLK)
            sbs.append(dict(
                idx_lo=_wrap16(lo_idx) if len(lo_idx) else
                    np.zeros((128, 0), np.int16),
                idx_hi=_wrap16(hi_idx) if len(hi_idx) else
                    np.zeros((128, 0), np.int16),
                colidx=np.ascontiguousarray(colidx.T),            # [128,nbk]
                srcid=srcid, dstid=dstid))
        cores.append(sbs)
    return nsb, sched, cores


def build_streams(cores, statv):
    """Per-edge a_src/a_dst expansion (indexing only) + packed stream blob."""
    asrcv, adstv = statv[:HEADS], statv[HEADS:]             # [4, n_all] f32
    outs = []
    for sbs in cores:
        blobs = []
        for sb in sbs:
            a_s = np.moveaxis(asrcv[:, sb["srcid"]], 0, -1)  # [nbk,128,4]
            a_d = np.moveaxis(adstv[:, sb["dstid"]], 0, -1)
            a8 = np.concatenate([a_s, a_d], axis=2)          # [nbk,128,8]
            a8 = np.ascontiguousarray(
                a8.transpose(1, 0, 2).astype(np.float16))    # [128,nbk,8]
            blobs.append(np.concatenate(
                [sb["idx_lo"], sb["idx_hi"], sb["colidx"].view(np.int16),
                 a8.reshape(128, -1).view(np.int16)], axis=1))
        outs.append(np.ascontiguousarray(np.concatenate(blobs, axis=1)))
    return outs


# -------------------------------------------------------------- device side

def build_nc2(cfg: Cfg, sched):
    nc = bacc.Bacc("TRN2", target_bir_lowering=False, debug=False,
                   num_devices=cfg.n_cores, num_swdge_queues=cfg.nq,
                   dynamic_dma_scratch_size=cfg.scratch)
    npc = cfg.npc
    HD = HEADS * OUT_DIM
    nsb = len(sched)
    nbk_s = [sum(l) + sum(h) for l, h in sched]
    TOT = sum(17 * b for b in nbk_s)
    nlo_max = max(sum(l) for l, _ in sched)
    nhi_max = max(sum(h) for _, h in sched)
    nbk_max = max(nbk_s)

    x_t = nc.dram_tensor("x16", [N_NODES, IN_DIM], F16, kind="ExternalInput")
    wf_t = nc.dram_tensor("wf", [IN_DIM, HD], F16, kind="ExternalInput")
    iota_t = nc.dram_tensor("iota16", [128, WIN], F16, kind="ExternalInput")
    biasb_t = nc.dram_tensor("biasb", [128, HD], F32, kind="ExternalInput")
    st_t = nc.dram_tensor("streams", [128, TOT], I16, kind="ExternalInput")
    out_t = nc.dram_tensor("out", [npc, HD], F32, kind="ExternalOutput")

    with tile.TileContext(nc) as tc:
        with tc.tile_pool(name="consts", bufs=1) as cpool:
            wf_c = cpool.tile([IN_DIM, HD], F16)
            nc.sync.dma_start(wf_c[:], wf_t.ap())
            iota_c = cpool.tile([128, WIN], F16)
            nc.sync.dma_start(iota_c[:], iota_t.ap())
            biasb_c = cpool.tile([128, HD], F32)
            nc.sync.dma_start(biasb_c[:], biasb_t.ap())
            ones_c = cpool.tile([128, 1], F16)
            nc.vector.memset(ones_c[:], 1.0)

            with (tc.tile_pool(name="pb_g", bufs=3) as gp,
                  tc.tile_pool(name="pb_m", bufs=2) as mp,
                  tc.tile_pool(name="pb_s", bufs=3) as sp,
                  tc.tile_pool(name="pb_z", bufs=2) as zp,
                  tc.tile_pool(name="pb_u", bufs=3, space="PSUM") as pu,
                  tc.tile_pool(name="pb_d", bufs=2, space="PSUM") as pdp,
                  tc.tile_pool(name="pb_w", bufs=2, space="PSUM") as pw):
                BL = cfg.blevel
                qi = 0
                for _rep in range(cfg.repeat):
                    off = 0
                    for sb in range(nsb):
                        nlo_l, nhi_l = sched[sb]
                        nlo, nhi = sum(nlo_l), sum(nhi_l)
                        nbk = nlo + nhi
                        W_sb = 17 * nbk
                        S0 = 8 * nlo
                        S1 = 8 * nbk
                        S2 = S1 + nbk
                        strm = sp.tile([128, 17 * nbk_max], I16, tag="strm")
                        nc.sync.dma_start(strm[:, :W_sb],
                                          st_t.ap()[:, off:off + W_sb])
                        off += W_sb
                        il = strm[:, 0:S0]
                        ih = strm[:, S0:S1]
                        cx = strm[:, S1:S2].bitcast(F16)
                        a8 = (strm[:, S2:W_sb].bitcast(F16)
                              .rearrange("p (b k) -> p b k", k=8))

                        glo = gp.tile([128, nlo_max, IN_DIM], F16, tag="glo")
                        for j0, nj in _gather_chunks(nlo * BLK, cfg.gnj):
                            nc.gpsimd.dma_gather(
                                glo[:, j0 // 128:(j0 + nj) // 128, :],
                                x_t.ap()[0:SPLIT, :],
                                il[:, j0 // 16:(j0 + nj) // 16],
                                nj, nj, IN_DIM, single_packet=cfg.sp,
                                queue_num=qi % cfg.nq)
                            qi += 1
                        ghi = gp.tile([128, nhi_max, IN_DIM], F16, tag="ghi")
                        for j0, nj in _gather_chunks(nhi * BLK, cfg.gnj):
                            nc.gpsimd.dma_gather(
                                ghi[:, j0 // 128:(j0 + nj) // 128, :],
                                x_t.ap()[SPLIT:N_NODES, :],
                                ih[:, j0 // 16:(j0 + nj) // 16],
                                nj, nj, IN_DIM, single_packet=cfg.sp,
                                queue_num=qi % cfg.nq)
                            qi += 1

                        if BL < 2:
                            continue
                        # expa = exp(lrelu(a_src + a_dst))
                        asum = sp.tile([128, nbk_max, HEADS], F32, tag="asum")
                        nc.vector.tensor_tensor(out=asum[:, :nbk, :],
                                                in0=a8[:, :, 0:HEADS],
                                                in1=a8[:, :, HEADS:8],
                                                op=mybir.AluOpType.add)
                        asc = sp.tile([128, nbk_max, HEADS], F32, tag="asc")
                        nc.vector.tensor_scalar(asc[:, :nbk, :],
                                                asum[:, :nbk, :], LEAKY_SLOPE,
                                                None, mybir.AluOpType.mult)
                        alr = sp.tile([128, nbk_max, HEADS], F32, tag="alr")
                        nc.vector.tensor_tensor(out=alr[:, :nbk, :],
                                                in0=asum[:, :nbk, :],
                                                in1=asc[:, :nbk, :],
                                                op=mybir.AluOpType.max)
                        expa = sp.tile([128, nbk_max, HEADS], F16, tag="expa")
                        nc.scalar.activation(expa[:, :nbk, :], alr[:, :nbk, :],
                                             mybir.ActivationFunctionType.Exp)
                        # onehot[e, b, c] = (iota[c] == colidx[e, b])
                        oneh = mp.tile([128, nbk_max, WIN], F16, tag="oneh")
                        nc.vector.tensor_tensor(
                            out=oneh[:, :nbk, :],
                            in0=iota_c[:].unsqueeze(1)
                                .to_broadcast([128, nbk, WIN]),
                            in1=cx.unsqueeze(2).to_broadcast([128, nbk, WIN]),
                            op=mybir.AluOpType.is_equal)
                        # Mw[e, b, h*32+c] = oneh * expa
                        mw = mp.tile([128, nbk_max, HEADS, WIN], F16, tag="mw")
                        nc.vector.tensor_tensor(
                            out=mw[:, :nbk, :, :],
                            in0=oneh[:, :nbk, :].unsqueeze(2)
                                .to_broadcast([128, nbk, HEADS, WIN]),
                            in1=expa[:, :nbk, :].unsqueeze(3)
                                .to_broadcast([128, nbk, HEADS, WIN]),
                            op=mybir.AluOpType.mult)

                        if BL < 3:
                            continue
                        dt_ps = pdp.tile([128, GPB], F32, tag="dt")
                        zall_ps = pw.tile([WIN, GPB, HEADS, OUT_DIM], F32,
                                          tag="zall")
                        lo_c = np.cumsum([0] + nlo_l)
                        hi_c = np.cumsum([0] + nhi_l)
                        for g in range(GPB):
                            blocks = (
                                [(glo, lo_c[g] + j, lo_c[g] + j)
                                 for j in range(nlo_l[g])]
                                + [(ghi, hi_c[g] + j, nlo + hi_c[g] + j)
                                   for j in range(nhi_l[g])])
                            u2 = pu.tile([128, HD], F32, tag="u2")
                            for k, (gt, slot, bcol) in enumerate(blocks):
                                st = k == 0
                                sp_ = k == len(blocks) - 1
                                nc.tensor.matmul(u2[:],
                                                 lhsT=gt[:, slot, :],
                                                 rhs=mw[:, bcol, :, :],
                                                 start=st, stop=sp_)
                                nc.tensor.matmul(dt_ps[:, g:g + 1],
                                                 lhsT=mw[:, bcol, :, :],
                                                 rhs=ones_c[:],
                                                 start=st, stop=sp_)
                            if BL < 4:
                                continue
                            u2s = zp.tile([128, HD], F16, tag="u2s")
                            nc.scalar.activation(
                                u2s[:], u2[:],
                                mybir.ActivationFunctionType.Copy)
                            for h in range(HEADS):
                                nc.tensor.matmul(
                                    zall_ps[:, g, h, :],
                                    lhsT=u2s[:, h * WIN:(h + 1) * WIN],
                                    rhs=wf_c[:, h * OUT_DIM:(h + 1) * OUT_DIM],
                                    start=True, stop=True)
                        if BL < 4:
                            continue
                        # denominators -> [c, g, h]; normalize; swish; store
                        dts = zp.tile([128, GPB], F32, tag="dts")
                        nc.vector.tensor_copy(dts[:], dt_ps[:])
                        rec = zp.tile([WIN, GPB, HEADS], F32, tag="rec")
                        for h in range(HEADS):
                            nc.vector.tensor_copy(
                                rec[:, :, h], dts[h * WIN:(h + 1) * WIN, :])
                        nc.vector.reciprocal(rec[:], rec[:])
                        zn = zp.tile([WIN, GPB, HEADS, OUT_DIM], F32, tag="zn")
                        nc.vector.tensor_tensor(
                            out=zn[:], in0=zall_ps[:],
                            in1=rec[:].unsqueeze(3)
                                .to_broadcast([WIN, GPB, HEADS, OUT_DIM]),
                            op=mybir.AluOpType.mult)
                        if cfg.bias_nonzero:
                            nc.vector.tensor_tensor(
                                out=zn[:], in0=zn[:],
                                in1=biasb_c[0:WIN, :]
                                    .rearrange("c (h o) -> c h o", o=OUT_DIM)
                                    .unsqueeze(1)
                                    .to_broadcast([WIN, GPB, HEADS, OUT_DIM]),
                                op=mybir.AluOpType.add)
                        sg = zp.tile([WIN, GPB, HEADS, OUT_DIM], F16, tag="sg")
                        nc.scalar.activation(
                            sg[:], zn[:], mybir.ActivationFunctionType.Sigmoid)
                        mix = zp.tile([WIN, GPB, HEADS, OUT_DIM], F16,
                                      tag="mix")
                        nc.vector.tensor_scalar(mix[:], sg[:], CMIX - BETA,
                                                BETA, mybir.AluOpType.mult,
                                                mybir.AluOpType.add)
                        zrow = zp.tile([WIN, GPB, HEADS, OUT_DIM], F32,
                                       tag="zrow")
                        nc.vector.tensor_tensor(out=zrow[:], in0=zn[:],
                                                in1=mix[:],
                                                op=mybir.AluOpType.mult)
                        n0 = sb * GPB * WIN
                        full = min(npc - n0, GPB * WIN)
                        ng = full // WIN
                        if ng:
                            nc.sync.dma_start(
                                out_t.ap()[n0:n0 + ng * WIN, :]
                                     .rearrange("(g c) (h o) -> c g h o",
                                                c=WIN, o=OUT_DIM),
                                zrow[:, 0:ng, :, :])
                        rem = full - ng * WIN
                        if rem:
                            nc.sync.dma_start(
                                out_t.ap()[n0 + ng * WIN:n0 + full, :]
                                     .rearrange("c (h o) -> c h o", o=OUT_DIM),
                                zrow[0:rem, ng, :, :])
    nc.compile()
    return nc


# ---------------------------------------------------------------- the API

def run(x, edge_index, W, att_src, att_dst, bias, trace=False, cfg_kw=None):
    npc = N_NODES // N_CORES
    nsb, sched, cores = preprocess(edge_index, N_NODES, npc, N_CORES)

    x16 = np.asarray(x, np.float32).astype(np.float16)      # [N, 128]
    W32 = np.asarray(W, dtype=np.float32)
    as32 = np.asarray(att_src, dtype=np.float32)
    ad32 = np.asarray(att_dst, dtype=np.float32)
    bias32 = np.asarray(bias, dtype=np.float32)
    S = np.zeros((HEADS * OUT_DIM, 2 * HEADS), dtype=np.float32)
    for h in range(HEADS):
        S[h * OUT_DIM:(h + 1) * OUT_DIM, h] = as32[h]
        S[h * OUT_DIM:(h + 1) * OUT_DIM, HEADS + h] = ad32[h]
    wad = (W32 @ S).astype(np.float16)          # param-only host matmul

    # launch 1: per-node attention stats
    nc1 = build_nc_stats(npc, N_CORES)
    in_maps1 = [dict(
        xT_slab=np.ascontiguousarray(x16[c * npc:(c + 1) * npc].T),
        wad_pd=wad) for c in range(N_CORES)]
    res1 = run_bass_kernel_spmd(nc1, in_maps1, core_ids=list(range(N_CORES)),
                                trace=trace)
    statv = np.concatenate(
        [res1.results[c]["statv"] for c in range(N_CORES)], axis=1)

    streams = build_streams(cores, statv)
    bias_nonzero = bool(np.any(bias32))
    kw = dict(cfg_kw or {})
    cfg = Cfg(npc, n_cores=N_CORES, bias_nonzero=bias_nonzero, **kw)

    nc2 = build_nc2(cfg, sched)
    iota = np.tile(np.arange(WIN, dtype=np.float16), (128, 1))
    biasb = np.tile(bias32, (128, 1)).astype(np.float32)
    wf16 = W32.astype(np.float16)
    in_maps = [dict(x16=x16, wf=wf16, iota16=iota, biasb=biasb,
                    streams=streams[c]) for c in range(N_CORES)]
    res = run_bass_kernel_spmd(nc2, in_maps, core_ids=list(range(N_CORES)),
                               trace=trace)
    out = np.concatenate([res.results[c]["out"] for c in range(N_CORES)],
                         axis=0)
    parts = dict(nc1=nc1, in_maps1=in_maps1, nc2=nc2, in_maps2=in_maps,
                 res1=res1, res2=res, n_cores=N_CORES, cfg=cfg, npc=npc,
                 sched=sched)
    return out, parts


def make_pjrt_fn(nc, in_maps, n_cores):
    """Build a jitted PJRT executor for a prebuilt Bass module (axon path).
    Returns (fn, args); inputs are pre-staged on device."""
    import jax
    from jax.sharding import Mesh, NamedSharding, PartitionSpec
    from jax.experimental.shard_map import shard_map

    import concourse.mybir as mybir_
    from concourse import bass2jax as b2j

    b2j.install_neuronx_cc_hook()
    partition_name = (nc.partition_id_tensor.name
                      if nc.partition_id_tensor else None)
    in_names, out_names, out_avals, zero_outs = [], [], [], []
    for alloc in nc.m.functions[0].allocations:
        if not isinstance(alloc, mybir_.MemoryLocationSet):
            continue
        name = alloc.memorylocations[0].name
        if alloc.kind == "ExternalInput":
            if name != partition_name:
                in_names.append(name)
        elif alloc.kind == "ExternalOutput":
            dt = mybir_.dt.np(alloc.dtype)
            out_avals.append(jax.core.ShapedArray(tuple(alloc.tensor_shape), dt))
            out_names.append(name)
            zero_outs.append(np.zeros(tuple(alloc.tensor_shape), dt))

    # the bind's in_names must cover ALL operands (inputs + zero-out bufs
    # + partition id) — neuronx_cc_hook asserts len(in_names) == n_operands.
    bind_names = list(in_names) + list(out_names)
    if partition_name is not None:
        bind_names.append(partition_name)

    def _body(*args):
        operands = list(args)
        if partition_name is not None:
            operands.append(b2j.partition_id_tensor())
        outs = b2j._bass_exec_p.bind(
            *operands, out_avals=tuple(out_avals), in_names=tuple(bind_names),
            out_names=tuple(out_names), lowering_input_output_aliases=(),
            sim_require_finite=True, sim_require_nnan=True, nc=nc)
        return tuple(outs)

    n_params = len(in_names)
    devices = jax.devices()[:n_cores]
    mesh = Mesh(np.asarray(devices), ("core",))
    spec = PartitionSpec("core")
    fn = jax.jit(shard_map(_body, mesh=mesh,
                           in_specs=(spec,) * (n_params + len(zero_outs)),
                           out_specs=(spec,) * len(out_names),
                           check_rep=False), keep_unused=True)
    sh = NamedSharding(mesh, spec)
    args = [jax.device_put(
                np.concatenate([in_maps[c][nm] for c in range(n_cores)], 0), sh)
            for nm in in_names]
    args += [jax.device_put(
                np.zeros((n_cores * z.shape[0], *z.shape[1:]), z.dtype), sh)
             for z in zero_outs]
    return fn, args


def bench_pair(fnA, argsA, fnB, argsB, iters=24):
    """Interleaved wall-clock of two executables with a 4-byte D2H fetch as
    the completion sync (block_until_ready alone is lost in ~40-90ms axon RPC
    noise; interleaving + cluster-min cancels the shared offset)."""
    import time as _time
    for fn, args in ((fnA, argsA), (fnB, argsB)):
        r = fn(*args)
        _ = np.asarray(r[0][0:1, 0:1])
    pa, pb = [], []
    for _ in range(iters):
        t0 = _time.perf_counter()
        r = fnA(*argsA)
        _ = np.asarray(r[0][0:1, 0:1])
        pa.append(_time.perf_counter() - t0)
        t0 = _time.perf_counter()
        r = fnB(*argsB)
        _ = np.asarray(r[0][0:1, 0:1])
        pb.append(_time.perf_counter() - t0)
    return np.array(pa), np.array(pb)


def bench_slope(ncA, ncB, in_maps, n_cores, reps, iters=24):
    """HW ns of one kernel body via the repeat-slope method: ncA has repeat=1,
    ncB has repeat=reps; returns (per-rep seconds, raw pair arrays)."""
    fnA, argsA = make_pjrt_fn(ncA, in_maps, n_cores)
    fnB, argsB = make_pjrt_fn(ncB, in_maps, n_cores)
    pa, pb = bench_pair(fnA, argsA, fnB, argsB, iters=iters)
    # medians: robust against the occasional anomalous fast RPC round-trip
    per = (np.median(pb) - np.median(pa)) / (reps - 1)
    return max(0.0, per), (pa, pb)


def kernel(**inputs) -> np.ndarray:
    out, _ = run(inputs["x"], inputs["edge_index"], inputs["W"],
                 inputs["att_src"], inputs["att_dst"], inputs["bias"])
    return out
